# revision 21
# baseline (speedup 1.0000x reference)
"""Trainium2 Bass kernel for a 5-layer MLP (512->256->128->64->32->512,
sigmoid on the first four layers) over batch 65536, data-parallel on 8 cores.

Contract: kernel(**inputs) takes the FULL unsharded inputs (np.ndarray, keyed
as in setup_inputs()) and returns the FULL [65536, 512] float32 output.

Strategy (v3):
  - shard the batch across 8 NeuronCores (8192 rows each), replicate weights
  - activations transposed on-chip (features on SBUF partitions); fp16
    compute with fp32 PSUM accumulation; x loaded as fp8e4m3 (input
    quantization is damped ~200x by the sigmoid stack; sim rel err 3.8e-4)
  - batch tiles processed in PAIRS (A|B, 512 cols each): matmuls N=512, but
    ACT/DVE/DMA ops span 1024 cols to amortize per-instruction overhead
  - tile_position packing: L3 col-tiled (A|B), L4 row-tiled with M widened
    to 128 so h4 comes out replicated for L5's four row-packed K=32 matmuls
  - SOFTWARE PIPELINING: pair t's dependency-chained tail layers are
    interleaved with pair t+1's L1 matmuls in emission order, so the PE
    queue never stalls on ACT results and the HAM clock gate stays warm
"""

import numpy as np
import ml_dtypes

import concourse.bass as bass
import concourse.mybir as mybir
import concourse.tile as tile
from concourse import bacc
from concourse.bass_utils import run_bass_kernel_spmd

N_CORES = 8
BATCH = 65536
B_C = BATCH // N_CORES  # 8192 rows per core
D_IN = 512
D_OUT = 512
NT = 512  # matmul free dim (one PSUM bank)
NP = 1024  # pair width: two adjacent batch tiles A|B
N_PAIRS = B_C // NP  # 8

W2_OFF = 1024
W3_OFF = 1280
W4_OFF = 1408
W5_OFF = 1536
W_COLS = 2048

_f8 = mybir.dt.float8e4
_f16 = mybir.dt.float16
_f32 = mybir.dt.float32


def _build_bass():
    # Bacc (not raw Bass): finalize() runs generate_event_semaphores, which
    # splits multi-sem waits into standalone EventSemaphore instructions --
    # TRN2 instructions can embed at most one sync wait.
    nc = bacc.Bacc(None)

    xt = nc.dram_tensor("xt", [D_IN, B_C], _f8, kind="ExternalInput")
    wp = nc.dram_tensor("wp", [128, W_COLS], _f16, kind="ExternalInput")
    bp = nc.dram_tensor("bp", [128, 9], _f32, kind="ExternalInput")
    yt = nc.dram_tensor("yt", [D_OUT, B_C], _f16, kind="ExternalOutput")

    sig = mybir.ActivationFunctionType.Sigmoid

    with tile.TileContext(nc) as tc:
        with (
            tc.tile_pool(name="consts", bufs=1) as consts,
            tc.tile_pool(name="xp", bufs=4) as xp,
            tc.tile_pool(name="hp", bufs=4) as hp,
            tc.tile_pool(name="yp", bufs=3) as yp,
            tc.tile_pool(name="ps1", bufs=1, space="PSUM") as ps1,
            tc.tile_pool(name="psm", bufs=1, space="PSUM") as psm,
            tc.tile_pool(name="ps5", bufs=2, space="PSUM") as ps5,
        ):
            # warm-up sigmoid with a single dependency: keeps the ACT
            # table-load off the first real (multi-dep) sigmoid
            warm = consts.tile([1, 2], _f32)
            nc.vector.memset(warm[:], 0.0)
            nc.scalar.activation(warm[:, 0:1], warm[:, 0:1], sig, bias=warm[:, 1:2])

            ws = consts.tile([128, W_COLS], _f16)
            bs = consts.tile([128, 9], _f32)

            def w1s(k, m):
                return ws[:, k * 256 + m * 128 : k * 256 + (m + 1) * 128]

            def w2s(k):
                return ws[:, W2_OFF + k * 128 : W2_OFF + (k + 1) * 128]

            xt3 = xt[:].rearrange("(ko p) n -> p ko n", p=128)  # [128, 4, B_C]
            yt3 = yt[:].rearrange("(mo p) n -> p mo n", p=128)  # [128, 4, B_C]

            halves = (slice(0, NT), slice(NT, NP))

            def load_x(t):
                xks = []
                for k in range(4):
                    xk = xp.tile([128, NP], _f8, tag=f"xk{k}", name=f"xk{k}_{t}")
                    nc.sync.dma_start(xk[:], xt3[:, k, bass.ts(t, NP)])
                    xks.append(xk)
                return xks

            def l1_mms(xks, m):
                # one m-chunk of L1 for a pair: 8 matmuls (ACT emitted later
                # so the tail-chain ACTs aren't head-of-line blocked)
                reg = ps1.tile([128, NP], _f32, tag="p1", name=f"p1_{m}")
                for h in halves:
                    for k in range(4):
                        nc.tensor.matmul(
                            reg[:, h], w1s(k, m), xks[k][:, h],
                            start=(k == 0), stop=(k == 3),
                        )
                return reg

            def l1_act(reg, m, h1):
                nc.scalar.activation(h1[:, m, :], reg[:], sig, bias=bs[:, m : m + 1])

            def l2_l4(h1):
                p2 = psm.tile([128, NP], _f32, tag="mid")
                for h in halves:
                    for k in range(2):
                        nc.tensor.matmul(
                            p2[:, h], w2s(k), h1[:, k, h], start=(k == 0), stop=(k == 1)
                        )
                h2 = hp.tile([128, NP], _f16, tag="h2")
                nc.scalar.activation(h2[:], p2[:], sig, bias=bs[:, 2:3])
                return h2

            def l3(h2):
                p3 = psm.tile([128, NT], _f32, tag="mid")
                nc.tensor.matmul(
                    p3[0:64, :], ws[:, W3_OFF : W3_OFF + 64], h2[:, halves[0]],
                    start=True, stop=True, tile_position=(0, 0),
                )
                nc.tensor.matmul(
                    p3[64:128, :], ws[:, W3_OFF + 64 : W3_OFF + 128], h2[:, halves[1]],
                    start=True, stop=True, tile_position=(0, 64),
                )
                h3 = hp.tile([128, NT], _f16, tag="h3")
                nc.scalar.activation(h3[:], p3[:], sig, bias=bs[:, 3:4])
                return h3

            def l4(h3):
                p4 = psm.tile([128, NP], _f32, tag="mid")
                nc.tensor.matmul(
                    p4[:, halves[0]], ws[0:64, W4_OFF : W4_OFF + 128], h3[0:64, :],
                    start=True, stop=True, tile_position=(0, 0),
                )
                nc.tensor.matmul(
                    p4[:, halves[1]], ws[64:128, W4_OFF : W4_OFF + 128], h3[64:128, :],
                    start=True, stop=True, tile_position=(64, 0),
                )
                h4 = hp.tile([128, NP], _f16, tag="h4")
                nc.scalar.activation(h4[:], p4[:], sig, bias=bs[:, 4:5])
                return h4

            def l5_chunk(h4, yts, m):
                p5 = ps5.tile([128, NP], _f32, tag="p5")
                for h in halves:
                    nc.tensor.matmul(
                        p5[:, h],
                        ws[32 * m : 32 * m + 32, W5_OFF + 128 * m : W5_OFF + 128 * (m + 1)],
                        h4[32 * m : 32 * m + 32, h],
                        start=True, stop=True, tile_position=(32 * m, 0),
                    )
                nc.vector.tensor_scalar_add(yts[:, m, :], p5[:], bs[:, 5 + m : 6 + m])

            # ---- software-pipelined emission ----
            # section t runs: tail chain of pair t (L2->L3->L4), L1 matmuls
            # of pair t+1 (PE filler), L5 of pair t-1, with pair t+1's L1
            # ACTs queued last so chain ACTs run with minimal queue delay
            xks0 = []
            for k in range(4):
                xk = xp.tile([128, NP], _f8, tag=f"xk{k}", name=f"xk{k}_0")
                nc.sync.dma_start(xk[:, 0:NT], xt3[:, k, 0:NT])
                xks0.append(xk)
            for k in range(4):
                nc.sync.dma_start(xks0[k][:, NT:NP], xt3[:, k, NT:NP])
            xks = {0: xks0}
            nc.sync.dma_start(ws[:], wp[:])
            nc.sync.dma_start(bs[:], bp[:])
            h1s = {}
            h4s = {}
            ytss = {}

            h1s[0] = hp.tile([128, 2, NP], _f16, tag="h1", name="h1_0")
            r0 = l1_mms(xks[0], 0)
            l1_act(r0, 0, h1s[0])
            r1 = l1_mms(xks[0], 1)
            l1_act(r1, 1, h1s[0])
            if N_PAIRS > 1:
                xks[1] = load_x(1)

            for t in range(N_PAIRS):
                if t + 2 < N_PAIRS:
                    xks[t + 2] = load_x(t + 2)

                h2 = l2_l4(h1s.pop(t))

                if t + 1 < N_PAIRS:
                    h1s[t + 1] = hp.tile([128, 2, NP], _f16, tag="h1", name=f"h1_{t+1}")
                    reg0 = l1_mms(xks[t + 1], 0)
                    l1_act(reg0, 0, h1s[t + 1])

                h3 = l3(h2)

                if t - 1 >= 0:
                    l5_chunk(h4s[t - 1], ytss[t - 1], 0)
                    l5_chunk(h4s[t - 1], ytss[t - 1], 1)
                    nc.sync.dma_start(
                        yt3[:, 0:2, bass.ts(t - 1, NP)], ytss[t - 1][:, 0:2, :]
                    )

                if t + 1 < N_PAIRS:
                    reg1 = l1_mms(xks.pop(t + 1), 1)
                    l1_act(reg1, 1, h1s[t + 1])

                h4s[t] = l4(h3)

                if t - 1 >= 0:
                    l5_chunk(h4s[t - 1], ytss[t - 1], 2)
                    l5_chunk(h4s.pop(t - 1), ytss[t - 1], 3)
                    nc.sync.dma_start(
                        yt3[:, 2:4, bass.ts(t - 1, NP)], ytss.pop(t - 1)[:, 2:4, :]
                    )

                ytss[t] = yp.tile([128, 4, NP], _f16, tag="yts", name=f"yts_{t}")

            t = N_PAIRS - 1
            l5_chunk(h4s[t], ytss[t], 0)
            nc.sync.dma_start(yt3[:, 0:1, bass.ts(t, NP)], ytss[t][:, 0:1, :])
            l5_chunk(h4s[t], ytss[t], 1)
            nc.sync.dma_start(yt3[:, 1:2, bass.ts(t, NP)], ytss[t][:, 1:2, :])
            l5_chunk(h4s[t], ytss[t], 2)
            nc.sync.dma_start(yt3[:, 2:3, bass.ts(t, NP)], ytss[t][:, 2:3, :])
            l5_chunk(h4s.pop(t), ytss[t], 3)
            nc.sync.dma_start(yt3[:, 3:4, bass.ts(t, NP)], ytss.pop(t)[:, 3:4, :])

    nc.finalize()
    return nc


_NC_CACHE = None


def _get_nc():
    global _NC_CACHE
    if _NC_CACHE is None:
        _NC_CACHE = _build_bass()
    return _NC_CACHE


def _pack_consts(w1, b1, w2, b2, w3, b3, w4, b4, w5, b5):
    wpk = np.zeros((128, W_COLS), dtype=np.float16)
    for k in range(4):
        wpk[:, k * 256 : (k + 1) * 256] = w1.T[k * 128 : (k + 1) * 128, :]
    for k in range(2):
        wpk[:, W2_OFF + k * 128 : W2_OFF + (k + 1) * 128] = w2.T[k * 128 : (k + 1) * 128, :]
    wpk[:, W3_OFF : W3_OFF + 64] = w3.T
    wpk[:, W3_OFF + 64 : W3_OFF + 128] = w3.T
    w4rep = np.tile(w4.T, (1, 4))  # [64, 128]
    wpk[0:64, W4_OFF : W4_OFF + 128] = w4rep
    wpk[64:128, W4_OFF : W4_OFF + 128] = w4rep
    for m in range(4):
        wpk[32 * m : 32 * m + 32, W5_OFF + 128 * m : W5_OFF + 128 * (m + 1)] = w5.T[
            :, 128 * m : 128 * (m + 1)
        ]

    bpk = np.zeros((128, 9), dtype=np.float32)
    bpk[:, 0] = b1[:128]
    bpk[:, 1] = b1[128:]
    bpk[:, 2] = b2
    bpk[0:64, 3] = b3
    bpk[64:128, 3] = b3
    bpk[:, 4] = np.tile(b4, 4)
    for m in range(4):
        bpk[:, 5 + m] = b5[m * 128 : (m + 1) * 128]
    return np.ascontiguousarray(wpk), np.ascontiguousarray(bpk)


def _make_in_maps(x, w1, b1, w2, b2, w3, b3, w4, b4, w5, b5):
    wpk, bpk = _pack_consts(w1, b1, w2, b2, w3, b3, w4, b4, w5, b5)
    shared = {"wp": wpk, "bp": bpk}
    in_maps = []
    for c in range(N_CORES):
        shard = x[c * B_C : (c + 1) * B_C]  # [B_C, 512]
        xtc = np.ascontiguousarray(shard.T.astype(ml_dtypes.float8_e4m3fn))
        in_maps.append({"xt": xtc, **shared})
    return in_maps


def _postprocess(x, results):
    y = np.empty((BATCH, D_OUT), dtype=np.float32)
    for c in range(N_CORES):
        y[c * B_C : (c + 1) * B_C] = results[c]["yt"].T.astype(np.float32)
    # reference: out[:, :in_size] = y, rest zero, in_size = count_nonzero(x[0])
    in_size = int(np.count_nonzero(x[0]))
    if in_size < D_OUT:
        y[:, in_size:] = 0.0
    return y


def run_traced(inputs, trace=False):
    """Run on 8 cores; returns (y_full, BassKernelResults)."""
    nc = _get_nc()
    in_maps = _make_in_maps(**inputs)
    res = run_bass_kernel_spmd(nc, in_maps, core_ids=list(range(N_CORES)), trace=trace)
    y = _postprocess(inputs["x"], res.results)
    return y, res


def kernel(**inputs) -> np.ndarray:
    y, _ = run_traced(inputs, trace=False)
    return y


# revision 23
# speedup vs baseline: 1.0703x; 1.0703x over previous
"""Trainium2 Bass kernel for a 5-layer MLP (512->256->128->64->32->512,
sigmoid on the first four layers) over batch 65536, data-parallel on 8 cores.

Contract: kernel(**inputs) takes the FULL unsharded inputs (np.ndarray, keyed
as in setup_inputs()) and returns the FULL [65536, 512] float32 output.

Strategy (v3):
  - shard the batch across 8 NeuronCores (8192 rows each), replicate weights
  - activations transposed on-chip (features on SBUF partitions); fp16
    compute with fp32 PSUM accumulation; x loaded as fp8e4m3 (input
    quantization is damped ~200x by the sigmoid stack; sim rel err 3.8e-4)
  - batch tiles processed in PAIRS (A|B, 512 cols each): matmuls N=512, but
    ACT/DVE/DMA ops span 1024 cols to amortize per-instruction overhead
  - tile_position packing: L3 col-tiled (A|B), L4 row-tiled with M widened
    to 128 so h4 comes out replicated for L5's four row-packed K=32 matmuls
  - SOFTWARE PIPELINING: pair t's dependency-chained tail layers are
    interleaved with pair t+1's L1 matmuls in emission order, so the PE
    queue never stalls on ACT results and the HAM clock gate stays warm
"""

import numpy as np
import ml_dtypes

import concourse.bass as bass
import concourse.mybir as mybir
import concourse.tile as tile
from concourse import bacc
from concourse.bass_utils import run_bass_kernel_spmd

N_CORES = 8
BATCH = 65536
B_C = BATCH // N_CORES  # 8192 rows per core
D_IN = 512
D_OUT = 512
NT = 512  # matmul free dim (one PSUM bank)
NP = 1024  # pair width: two adjacent batch tiles A|B
N_PAIRS = B_C // NP  # 8

W2_OFF = 1024
W3_OFF = 1280
W4_OFF = 1408
W5_OFF = 1536
W_COLS = 2048

_f8 = mybir.dt.float8e4
_f16 = mybir.dt.float16
_f32 = mybir.dt.float32


def _build_bass():
    # Bacc (not raw Bass): finalize() runs generate_event_semaphores, which
    # splits multi-sem waits into standalone EventSemaphore instructions --
    # TRN2 instructions can embed at most one sync wait.
    nc = bacc.Bacc(None)

    xt = nc.dram_tensor("xt", [D_IN, B_C], _f8, kind="ExternalInput")
    wp = nc.dram_tensor("wp", [128, W_COLS], _f16, kind="ExternalInput")
    wd = nc.dram_tensor("wd", [128, 1024], _f8, kind="ExternalInput")
    bp = nc.dram_tensor("bp", [128, 9], _f32, kind="ExternalInput")
    yt = nc.dram_tensor("yt", [D_OUT, B_C], _f16, kind="ExternalOutput")

    sig = mybir.ActivationFunctionType.Sigmoid

    with tile.TileContext(nc) as tc:
        with (
            tc.tile_pool(name="consts", bufs=1) as consts,
            tc.tile_pool(name="xp", bufs=4) as xp,
            tc.tile_pool(name="hp", bufs=4) as hp,
            tc.tile_pool(name="yp", bufs=3) as yp,
            tc.tile_pool(name="ps1", bufs=1, space="PSUM") as ps1,
            tc.tile_pool(name="psm", bufs=1, space="PSUM") as psm,
            tc.tile_pool(name="ps5", bufs=2, space="PSUM") as ps5,
        ):
            # warm-up sigmoid with a single dependency: keeps the ACT
            # table-load off the first real (multi-dep) sigmoid
            warm = consts.tile([1, 2], _f32)
            nc.vector.memset(warm[:], 0.0)
            nc.scalar.activation(warm[:, 0:1], warm[:, 0:1], sig, bias=warm[:, 1:2])

            ws = consts.tile([128, W_COLS], _f16)
            wds = consts.tile([128, 1024], _f8)
            bs = consts.tile([128, 9], _f32)

            def w1s(k, m):
                return ws[:, k * 256 + m * 128 : k * 256 + (m + 1) * 128]

            def w2s(k):
                return ws[:, W2_OFF + k * 128 : W2_OFF + (k + 1) * 128]

            xt3 = xt[:].rearrange("(ko p) n -> p ko n", p=128)  # [128, 4, B_C]
            yt3 = yt[:].rearrange("(mo p) n -> p mo n", p=128)  # [128, 4, B_C]

            halves = (slice(0, NT), slice(NT, NP))

            def load_x(t):
                # two [128, 2, NP] tiles: DoubleRow pairs two 128-feature
                # rows per PE cell, contracting 256 features per matmul
                xgs = []
                for g in range(2):
                    xg = xp.tile([128, 2, NP], _f8, tag=f"xg{g}", name=f"xg{g}_{t}")
                    nc.sync.dma_start(xg[:], xt3[:, 2 * g : 2 * g + 2, bass.ts(t, NP)])
                    xgs.append(xg)
                return xgs

            def l1_mms(xgs, m):
                # one m-chunk of L1 for a pair: 4 fp8 DoubleRow matmuls
                # (K=256 each), ACT emitted later
                reg = ps1.tile([128, NP], _f32, tag="p1", name=f"p1_{m}")
                for h in halves:
                    for g in range(2):
                        bb = (g * 2 + m) * 256
                        nc.tensor.matmul(
                            reg[:, h],
                            wds[:, bb : bb + 256].rearrange("p (j mm) -> p j mm", j=2),
                            xgs[g][:, :, h],
                            start=(g == 0), stop=(g == 1),
                            perf_mode=mybir.MatmulPerfMode.DoubleRow,
                        )
                return reg

            def l1_act(reg, m, h1):
                nc.scalar.activation(h1[:, m, :], reg[:], sig, bias=bs[:, m : m + 1])

            def l2_l4(h1):
                p2 = psm.tile([128, NP], _f32, tag="mid")
                for h in halves:
                    for k in range(2):
                        nc.tensor.matmul(
                            p2[:, h], w2s(k), h1[:, k, h], start=(k == 0), stop=(k == 1)
                        )
                h2 = hp.tile([128, NP], _f16, tag="h2")
                nc.scalar.activation(h2[:], p2[:], sig, bias=bs[:, 2:3])
                return h2

            def l3(h2):
                p3 = psm.tile([128, NT], _f32, tag="mid")
                nc.tensor.matmul(
                    p3[0:64, :], ws[:, W3_OFF : W3_OFF + 64], h2[:, halves[0]],
                    start=True, stop=True, tile_position=(0, 0),
                )
                nc.tensor.matmul(
                    p3[64:128, :], ws[:, W3_OFF + 64 : W3_OFF + 128], h2[:, halves[1]],
                    start=True, stop=True, tile_position=(0, 64),
                )
                h3 = hp.tile([128, NT], _f16, tag="h3")
                nc.scalar.activation(h3[:], p3[:], sig, bias=bs[:, 3:4])
                return h3

            def l4(h3):
                p4 = psm.tile([128, NP], _f32, tag="mid")
                nc.tensor.matmul(
                    p4[:, halves[0]], ws[0:64, W4_OFF : W4_OFF + 128], h3[0:64, :],
                    start=True, stop=True, tile_position=(0, 0),
                )
                nc.tensor.matmul(
                    p4[:, halves[1]], ws[64:128, W4_OFF : W4_OFF + 128], h3[64:128, :],
                    start=True, stop=True, tile_position=(64, 0),
                )
                h4 = hp.tile([128, NP], _f16, tag="h4")
                nc.scalar.activation(h4[:], p4[:], sig, bias=bs[:, 4:5])
                return h4

            def l5_chunk(h4, yts, m):
                p5 = ps5.tile([128, NP], _f32, tag="p5")
                for h in halves:
                    nc.tensor.matmul(
                        p5[:, h],
                        ws[32 * m : 32 * m + 32, W5_OFF + 128 * m : W5_OFF + 128 * (m + 1)],
                        h4[32 * m : 32 * m + 32, h],
                        start=True, stop=True, tile_position=(32 * m, 0),
                    )
                nc.vector.tensor_scalar_add(yts[:, m, :], p5[:], bs[:, 5 + m : 6 + m])

            # ---- software-pipelined emission ----
            # section t runs: tail chain of pair t (L2->L3->L4), L1 matmuls
            # of pair t+1 (PE filler), L5 of pair t-1, with pair t+1's L1
            # ACTs queued last so chain ACTs run with minimal queue delay
            xks = {0: load_x(0)}
            nc.sync.dma_start(ws[:], wp[:])
            nc.sync.dma_start(wds[:], wd[:])
            nc.sync.dma_start(bs[:], bp[:])
            h1s = {}
            h4s = {}
            ytss = {}

            h1s[0] = hp.tile([128, 2, NP], _f16, tag="h1", name="h1_0")
            r0 = l1_mms(xks[0], 0)
            l1_act(r0, 0, h1s[0])
            r1 = l1_mms(xks[0], 1)
            l1_act(r1, 1, h1s[0])
            if N_PAIRS > 1:
                xks[1] = load_x(1)

            for t in range(N_PAIRS):
                if t + 2 < N_PAIRS:
                    xks[t + 2] = load_x(t + 2)

                h2 = l2_l4(h1s.pop(t))

                if t + 1 < N_PAIRS:
                    h1s[t + 1] = hp.tile([128, 2, NP], _f16, tag="h1", name=f"h1_{t+1}")
                    reg0 = l1_mms(xks[t + 1], 0)
                    l1_act(reg0, 0, h1s[t + 1])

                h3 = l3(h2)

                if t + 1 < N_PAIRS:
                    reg1 = l1_mms(xks.pop(t + 1), 1)
                    l1_act(reg1, 1, h1s[t + 1])

                if t - 1 >= 0:
                    l5_chunk(h4s[t - 1], ytss[t - 1], 0)
                    l5_chunk(h4s[t - 1], ytss[t - 1], 1)
                    nc.sync.dma_start(
                        yt3[:, 0:2, bass.ts(t - 1, NP)], ytss[t - 1][:, 0:2, :]
                    )

                h4s[t] = l4(h3)

                if t - 1 >= 0:
                    l5_chunk(h4s[t - 1], ytss[t - 1], 2)
                    l5_chunk(h4s.pop(t - 1), ytss[t - 1], 3)
                    nc.sync.dma_start(
                        yt3[:, 2:4, bass.ts(t - 1, NP)], ytss.pop(t - 1)[:, 2:4, :]
                    )

                ytss[t] = yp.tile([128, 4, NP], _f16, tag="yts", name=f"yts_{t}")

            t = N_PAIRS - 1
            l5_chunk(h4s[t], ytss[t], 0)
            nc.sync.dma_start(yt3[:, 0:1, bass.ts(t, NP)], ytss[t][:, 0:1, :])
            l5_chunk(h4s[t], ytss[t], 1)
            nc.sync.dma_start(yt3[:, 1:2, bass.ts(t, NP)], ytss[t][:, 1:2, :])
            l5_chunk(h4s[t], ytss[t], 2)
            nc.sync.dma_start(yt3[:, 2:3, bass.ts(t, NP)], ytss[t][:, 2:3, :])
            l5_chunk(h4s.pop(t), ytss[t], 3)
            nc.sync.dma_start(yt3[:, 3:4, bass.ts(t, NP)], ytss.pop(t)[:, 3:4, :])

    nc.finalize()
    return nc


_NC_CACHE = None


def _get_nc():
    global _NC_CACHE
    if _NC_CACHE is None:
        _NC_CACHE = _build_bass()
    return _NC_CACHE


def _pack_consts(w1, b1, w2, b2, w3, b3, w4, b4, w5, b5):
    wpk = np.zeros((128, W_COLS), dtype=np.float16)
    for k in range(4):
        wpk[:, k * 256 : (k + 1) * 256] = w1.T[k * 128 : (k + 1) * 128, :]
    for k in range(2):
        wpk[:, W2_OFF + k * 128 : W2_OFF + (k + 1) * 128] = w2.T[k * 128 : (k + 1) * 128, :]
    wpk[:, W3_OFF : W3_OFF + 64] = w3.T
    wpk[:, W3_OFF + 64 : W3_OFF + 128] = w3.T
    w4rep = np.tile(w4.T, (1, 4))  # [64, 128]
    wpk[0:64, W4_OFF : W4_OFF + 128] = w4rep
    wpk[64:128, W4_OFF : W4_OFF + 128] = w4rep
    for m in range(4):
        wpk[32 * m : 32 * m + 32, W5_OFF + 128 * m : W5_OFF + 128 * (m + 1)] = w5.T[
            :, 128 * m : 128 * (m + 1)
        ]

    bpk = np.zeros((128, 9), dtype=np.float32)
    bpk[:, 0] = b1[:128]
    bpk[:, 1] = b1[128:]
    bpk[:, 2] = b2
    bpk[0:64, 3] = b3
    bpk[64:128, 3] = b3
    bpk[:, 4] = np.tile(b4, 4)
    for m in range(4):
        bpk[:, 5 + m] = b5[m * 128 : (m + 1) * 128]
    return np.ascontiguousarray(wpk), np.ascontiguousarray(bpk)


def _pack_w1_dr(w1):
    wdk = np.zeros((128, 1024), dtype=np.float32)
    for g in range(2):
        for m in range(2):
            base = (g * 2 + m) * 256
            for j in range(2):
                wdk[:, base + j * 128 : base + (j + 1) * 128] = w1[
                    m * 128 : (m + 1) * 128, g * 256 + j * 128 : g * 256 + (j + 1) * 128
                ].T
    return np.ascontiguousarray(wdk.astype(ml_dtypes.float8_e4m3fn))


def _make_in_maps(x, w1, b1, w2, b2, w3, b3, w4, b4, w5, b5):
    wpk, bpk = _pack_consts(w1, b1, w2, b2, w3, b3, w4, b4, w5, b5)
    shared = {"wp": wpk, "bp": bpk, "wd": _pack_w1_dr(w1)}
    in_maps = []
    for c in range(N_CORES):
        shard = x[c * B_C : (c + 1) * B_C]  # [B_C, 512]
        xtc = np.ascontiguousarray(shard.T.astype(ml_dtypes.float8_e4m3fn))
        in_maps.append({"xt": xtc, **shared})
    return in_maps


def _postprocess(x, results):
    y = np.empty((BATCH, D_OUT), dtype=np.float32)
    for c in range(N_CORES):
        y[c * B_C : (c + 1) * B_C] = results[c]["yt"].T.astype(np.float32)
    # reference: out[:, :in_size] = y, rest zero, in_size = count_nonzero(x[0])
    in_size = int(np.count_nonzero(x[0]))
    if in_size < D_OUT:
        y[:, in_size:] = 0.0
    return y


def run_traced(inputs, trace=False):
    """Run on 8 cores; returns (y_full, BassKernelResults)."""
    nc = _get_nc()
    in_maps = _make_in_maps(**inputs)
    res = run_bass_kernel_spmd(nc, in_maps, core_ids=list(range(N_CORES)), trace=trace)
    y = _postprocess(inputs["x"], res.results)
    return y, res


def kernel(**inputs) -> np.ndarray:
    y, _ = run_traced(inputs, trace=False)
    return y


# revision 24
# speedup vs baseline: 1.1009x; 1.0286x over previous
"""Trainium2 Bass kernel for a 5-layer MLP (512->256->128->64->32->512,
sigmoid on the first four layers) over batch 65536, data-parallel on 8 cores.

Contract: kernel(**inputs) takes the FULL unsharded inputs (np.ndarray, keyed
as in setup_inputs()) and returns the FULL [65536, 512] float32 output.

Strategy (v3):
  - shard the batch across 8 NeuronCores (8192 rows each), replicate weights
  - activations transposed on-chip (features on SBUF partitions); fp16
    compute with fp32 PSUM accumulation; x loaded as fp8e4m3 (input
    quantization is damped ~200x by the sigmoid stack; sim rel err 3.8e-4)
  - batch tiles processed in PAIRS (A|B, 512 cols each): matmuls N=512, but
    ACT/DVE/DMA ops span 1024 cols to amortize per-instruction overhead
  - tile_position packing: L3 col-tiled (A|B), L4 row-tiled with M widened
    to 128 so h4 comes out replicated for L5's four row-packed K=32 matmuls
  - SOFTWARE PIPELINING: pair t's dependency-chained tail layers are
    interleaved with pair t+1's L1 matmuls in emission order, so the PE
    queue never stalls on ACT results and the HAM clock gate stays warm
"""

import numpy as np
import ml_dtypes

import concourse.bass as bass
import concourse.mybir as mybir
import concourse.tile as tile
from concourse import bacc
from concourse.bass_utils import run_bass_kernel_spmd

N_CORES = 8
BATCH = 65536
B_C = BATCH // N_CORES  # 8192 rows per core
D_IN = 512
D_OUT = 512
NT = 512  # matmul free dim (one PSUM bank)
NP = 1024  # pair width: two adjacent batch tiles A|B
N_PAIRS = B_C // NP  # 8

W2_OFF = 1024
W3_OFF = 1280
W4_OFF = 1408
W5_OFF = 1536
W_COLS = 2048

_f8 = mybir.dt.float8e4
_f16 = mybir.dt.float16
_f32 = mybir.dt.float32


def _build_bass():
    # Bacc (not raw Bass): finalize() runs generate_event_semaphores, which
    # splits multi-sem waits into standalone EventSemaphore instructions --
    # TRN2 instructions can embed at most one sync wait.
    nc = bacc.Bacc(None)

    xt = nc.dram_tensor("xt", [D_IN, B_C], _f8, kind="ExternalInput")
    wp = nc.dram_tensor("wp", [128, W_COLS], _f16, kind="ExternalInput")
    wd = nc.dram_tensor("wd", [128, 1024], _f8, kind="ExternalInput")
    bp = nc.dram_tensor("bp", [128, 9], _f32, kind="ExternalInput")
    yt = nc.dram_tensor("yt", [D_OUT, B_C], _f16, kind="ExternalOutput")

    sig = mybir.ActivationFunctionType.Sigmoid

    with tile.TileContext(nc) as tc:
        with (
            tc.tile_pool(name="consts", bufs=1) as consts,
            tc.tile_pool(name="xp", bufs=4) as xp,
            tc.tile_pool(name="hp", bufs=4) as hp,
            tc.tile_pool(name="yp", bufs=3) as yp,
            tc.tile_pool(name="ps1", bufs=1, space="PSUM") as ps1,
            tc.tile_pool(name="psm", bufs=1, space="PSUM") as psm,
            tc.tile_pool(name="ps5", bufs=2, space="PSUM") as ps5,
        ):
            # warm-up sigmoid with a single dependency: keeps the ACT
            # table-load off the first real (multi-dep) sigmoid
            warm = consts.tile([1, 2], _f32)
            nc.vector.memset(warm[:], 0.0)
            nc.scalar.activation(warm[:, 0:1], warm[:, 0:1], sig, bias=warm[:, 1:2])

            ws = consts.tile([128, W_COLS], _f16)
            wds = consts.tile([128, 1024], _f8)
            bs = consts.tile([128, 9], _f32)

            def w1s(k, m):
                return ws[:, k * 256 + m * 128 : k * 256 + (m + 1) * 128]

            def w2s(k):
                return ws[:, W2_OFF + k * 128 : W2_OFF + (k + 1) * 128]

            xt3 = xt[:].rearrange("(ko p) n -> p ko n", p=128)  # [128, 4, B_C]
            yt3 = yt[:].rearrange("(mo p) n -> p mo n", p=128)  # [128, 4, B_C]

            halves = (slice(0, NT), slice(NT, NP))

            def load_x(t):
                # two [128, 2, NP] tiles: DoubleRow pairs two 128-feature
                # rows per PE cell, contracting 256 features per matmul
                xgs = []
                for g in range(2):
                    xg = xp.tile([128, 2, NP], _f8, tag=f"xg{g}", name=f"xg{g}_{t}")
                    nc.sync.dma_start(xg[:], xt3[:, 2 * g : 2 * g + 2, bass.ts(t, NP)])
                    xgs.append(xg)
                return xgs

            def l1_mms(xgs, m):
                # one m-chunk of L1 for a pair: 4 fp8 DoubleRow matmuls
                # (K=256 each), ACT emitted later
                reg = ps1.tile([128, NP], _f32, tag="p1", name=f"p1_{m}")
                for h in halves:
                    for g in range(2):
                        bb = (g * 2 + m) * 256
                        nc.tensor.matmul(
                            reg[:, h],
                            wds[:, bb : bb + 256].rearrange("p (j mm) -> p j mm", j=2),
                            xgs[g][:, :, h],
                            start=(g == 0), stop=(g == 1),
                            perf_mode=mybir.MatmulPerfMode.DoubleRow,
                        )
                return reg

            def l1_act(reg, m, h1):
                nc.scalar.activation(h1[:, m, :], reg[:], sig, bias=bs[:, m : m + 1])

            def l2_l4(h1):
                p2 = psm.tile([128, NP], _f32, tag="mid")
                for h in halves:
                    for k in range(2):
                        nc.tensor.matmul(
                            p2[:, h], w2s(k), h1[:, k, h], start=(k == 0), stop=(k == 1)
                        )
                h2 = hp.tile([128, NP], _f16, tag="h2")
                nc.scalar.activation(h2[:], p2[:], sig, bias=bs[:, 2:3])
                return h2

            def l3(h2):
                p3 = psm.tile([128, NT], _f32, tag="mid")
                nc.tensor.matmul(
                    p3[0:64, :], ws[:, W3_OFF : W3_OFF + 64], h2[:, halves[0]],
                    start=True, stop=True, tile_position=(0, 0),
                )
                nc.tensor.matmul(
                    p3[64:128, :], ws[:, W3_OFF + 64 : W3_OFF + 128], h2[:, halves[1]],
                    start=True, stop=True, tile_position=(0, 64),
                )
                h3 = hp.tile([128, NT], _f16, tag="h3")
                nc.scalar.activation(h3[:], p3[:], sig, bias=bs[:, 3:4])
                return h3

            def l4(h3):
                p4 = psm.tile([128, NP], _f32, tag="mid")
                nc.tensor.matmul(
                    p4[:, halves[0]], ws[0:64, W4_OFF : W4_OFF + 128], h3[0:64, :],
                    start=True, stop=True, tile_position=(0, 0),
                )
                nc.tensor.matmul(
                    p4[:, halves[1]], ws[64:128, W4_OFF : W4_OFF + 128], h3[64:128, :],
                    start=True, stop=True, tile_position=(64, 0),
                )
                h4 = hp.tile([128, NP], _f16, tag="h4")
                nc.scalar.activation(h4[:], p4[:], sig, bias=bs[:, 4:5])
                return h4

            def l5_chunk(h4, yts, m):
                p5 = ps5.tile([128, NP], _f32, tag="p5")
                for h in halves:
                    nc.tensor.matmul(
                        p5[:, h],
                        ws[32 * m : 32 * m + 32, W5_OFF + 128 * m : W5_OFF + 128 * (m + 1)],
                        h4[32 * m : 32 * m + 32, h],
                        start=True, stop=True, tile_position=(32 * m, 0),
                    )
                nc.vector.tensor_scalar_add(yts[:, m, :], p5[:], bs[:, 5 + m : 6 + m])

            # ---- software-pipelined emission ----
            # section t runs: tail chain of pair t (L2->L3->L4), L1 matmuls
            # of pair t+1 (PE filler), L5 of pair t-1, with pair t+1's L1
            # ACTs queued last so chain ACTs run with minimal queue delay
            xks = {0: load_x(0)}
            nc.sync.dma_start(wds[:], wd[:])
            nc.sync.dma_start(bs[:], bp[:])
            nc.sync.dma_start(ws[:], wp[:])
            h1s = {}
            h4s = {}
            ytss = {}

            h1s[0] = hp.tile([128, 2, NP], _f16, tag="h1", name="h1_0")
            r0 = l1_mms(xks[0], 0)
            l1_act(r0, 0, h1s[0])
            r1 = l1_mms(xks[0], 1)
            l1_act(r1, 1, h1s[0])
            if N_PAIRS > 1:
                xks[1] = load_x(1)

            for t in range(N_PAIRS):
                if t + 2 < N_PAIRS:
                    xks[t + 2] = load_x(t + 2)

                h2 = l2_l4(h1s.pop(t))

                if t + 1 < N_PAIRS:
                    h1s[t + 1] = hp.tile([128, 2, NP], _f16, tag="h1", name=f"h1_{t+1}")
                    reg0 = l1_mms(xks[t + 1], 0)
                    l1_act(reg0, 0, h1s[t + 1])

                h3 = l3(h2)

                if t + 1 < N_PAIRS:
                    reg1 = l1_mms(xks.pop(t + 1), 1)
                    l1_act(reg1, 1, h1s[t + 1])

                if t - 1 >= 0:
                    l5_chunk(h4s[t - 1], ytss[t - 1], 0)
                    l5_chunk(h4s[t - 1], ytss[t - 1], 1)
                    nc.sync.dma_start(
                        yt3[:, 0:2, bass.ts(t - 1, NP)], ytss[t - 1][:, 0:2, :]
                    )
                    l5_chunk(h4s[t - 1], ytss[t - 1], 2)
                    l5_chunk(h4s.pop(t - 1), ytss[t - 1], 3)
                    nc.sync.dma_start(
                        yt3[:, 2:4, bass.ts(t - 1, NP)], ytss.pop(t - 1)[:, 2:4, :]
                    )

                h4s[t] = l4(h3)

                ytss[t] = yp.tile([128, 4, NP], _f16, tag="yts", name=f"yts_{t}")

            t = N_PAIRS - 1
            l5_chunk(h4s[t], ytss[t], 0)
            nc.sync.dma_start(yt3[:, 0:1, bass.ts(t, NP)], ytss[t][:, 0:1, :])
            l5_chunk(h4s[t], ytss[t], 1)
            nc.sync.dma_start(yt3[:, 1:2, bass.ts(t, NP)], ytss[t][:, 1:2, :])
            l5_chunk(h4s[t], ytss[t], 2)
            nc.sync.dma_start(yt3[:, 2:3, bass.ts(t, NP)], ytss[t][:, 2:3, :])
            l5_chunk(h4s.pop(t), ytss[t], 3)
            nc.sync.dma_start(yt3[:, 3:4, bass.ts(t, NP)], ytss.pop(t)[:, 3:4, :])

    nc.finalize()
    return nc


_NC_CACHE = None


def _get_nc():
    global _NC_CACHE
    if _NC_CACHE is None:
        _NC_CACHE = _build_bass()
    return _NC_CACHE


def _pack_consts(w1, b1, w2, b2, w3, b3, w4, b4, w5, b5):
    wpk = np.zeros((128, W_COLS), dtype=np.float16)
    for k in range(4):
        wpk[:, k * 256 : (k + 1) * 256] = w1.T[k * 128 : (k + 1) * 128, :]
    for k in range(2):
        wpk[:, W2_OFF + k * 128 : W2_OFF + (k + 1) * 128] = w2.T[k * 128 : (k + 1) * 128, :]
    wpk[:, W3_OFF : W3_OFF + 64] = w3.T
    wpk[:, W3_OFF + 64 : W3_OFF + 128] = w3.T
    w4rep = np.tile(w4.T, (1, 4))  # [64, 128]
    wpk[0:64, W4_OFF : W4_OFF + 128] = w4rep
    wpk[64:128, W4_OFF : W4_OFF + 128] = w4rep
    for m in range(4):
        wpk[32 * m : 32 * m + 32, W5_OFF + 128 * m : W5_OFF + 128 * (m + 1)] = w5.T[
            :, 128 * m : 128 * (m + 1)
        ]

    bpk = np.zeros((128, 9), dtype=np.float32)
    bpk[:, 0] = b1[:128]
    bpk[:, 1] = b1[128:]
    bpk[:, 2] = b2
    bpk[0:64, 3] = b3
    bpk[64:128, 3] = b3
    bpk[:, 4] = np.tile(b4, 4)
    for m in range(4):
        bpk[:, 5 + m] = b5[m * 128 : (m + 1) * 128]
    return np.ascontiguousarray(wpk), np.ascontiguousarray(bpk)


def _pack_w1_dr(w1):
    wdk = np.zeros((128, 1024), dtype=np.float32)
    for g in range(2):
        for m in range(2):
            base = (g * 2 + m) * 256
            for j in range(2):
                wdk[:, base + j * 128 : base + (j + 1) * 128] = w1[
                    m * 128 : (m + 1) * 128, g * 256 + j * 128 : g * 256 + (j + 1) * 128
                ].T
    return np.ascontiguousarray(wdk.astype(ml_dtypes.float8_e4m3fn))


def _make_in_maps(x, w1, b1, w2, b2, w3, b3, w4, b4, w5, b5):
    wpk, bpk = _pack_consts(w1, b1, w2, b2, w3, b3, w4, b4, w5, b5)
    shared = {"wp": wpk, "bp": bpk, "wd": _pack_w1_dr(w1)}
    in_maps = []
    for c in range(N_CORES):
        shard = x[c * B_C : (c + 1) * B_C]  # [B_C, 512]
        xtc = np.ascontiguousarray(shard.T.astype(ml_dtypes.float8_e4m3fn))
        in_maps.append({"xt": xtc, **shared})
    return in_maps


def _postprocess(x, results):
    y = np.empty((BATCH, D_OUT), dtype=np.float32)
    for c in range(N_CORES):
        y[c * B_C : (c + 1) * B_C] = results[c]["yt"].T.astype(np.float32)
    # reference: out[:, :in_size] = y, rest zero, in_size = count_nonzero(x[0])
    in_size = int(np.count_nonzero(x[0]))
    if in_size < D_OUT:
        y[:, in_size:] = 0.0
    return y


def run_traced(inputs, trace=False):
    """Run on 8 cores; returns (y_full, BassKernelResults)."""
    nc = _get_nc()
    in_maps = _make_in_maps(**inputs)
    res = run_bass_kernel_spmd(nc, in_maps, core_ids=list(range(N_CORES)), trace=trace)
    y = _postprocess(inputs["x"], res.results)
    return y, res


def kernel(**inputs) -> np.ndarray:
    y, _ = run_traced(inputs, trace=False)
    return y


# revision 25
# speedup vs baseline: 1.1518x; 1.0463x over previous
"""Trainium2 Bass kernel for a 5-layer MLP (512->256->128->64->32->512,
sigmoid on the first four layers) over batch 65536, data-parallel on 8 cores.

Contract: kernel(**inputs) takes the FULL unsharded inputs (np.ndarray, keyed
as in setup_inputs()) and returns the FULL [65536, 512] float32 output.

Strategy (v3):
  - shard the batch across 8 NeuronCores (8192 rows each), replicate weights
  - activations transposed on-chip (features on SBUF partitions); fp16
    compute with fp32 PSUM accumulation; x loaded as fp8e4m3 (input
    quantization is damped ~200x by the sigmoid stack; sim rel err 3.8e-4)
  - batch tiles processed in PAIRS (A|B, 512 cols each): matmuls N=512, but
    ACT/DVE/DMA ops span 1024 cols to amortize per-instruction overhead
  - tile_position packing: L3 col-tiled (A|B), L4 row-tiled with M widened
    to 128 so h4 comes out replicated for L5's four row-packed K=32 matmuls
  - SOFTWARE PIPELINING: pair t's dependency-chained tail layers are
    interleaved with pair t+1's L1 matmuls in emission order, so the PE
    queue never stalls on ACT results and the HAM clock gate stays warm
"""

import numpy as np
import ml_dtypes

import concourse.bass as bass
import concourse.mybir as mybir
import concourse.tile as tile
from concourse import bacc
from concourse.bass_utils import run_bass_kernel_spmd

N_CORES = 8
BATCH = 65536
B_C = BATCH // N_CORES  # 8192 rows per core
D_IN = 512
D_OUT = 512
NT = 512  # matmul free dim (one PSUM bank)
NP = 1024  # pair width: two adjacent batch tiles A|B
N_PAIRS = B_C // NP  # 8

W2_OFF = 1024
W3_OFF = 1280
W4_OFF = 1408
W5_OFF = 1536
W_COLS = 2048

_f8 = mybir.dt.float8e4
_f16 = mybir.dt.float16
_f32 = mybir.dt.float32


def _build_bass():
    # Bacc (not raw Bass): finalize() runs generate_event_semaphores, which
    # splits multi-sem waits into standalone EventSemaphore instructions --
    # TRN2 instructions can embed at most one sync wait.
    nc = bacc.Bacc(None)

    xt = nc.dram_tensor("xt", [D_IN, B_C], _f8, kind="ExternalInput")
    wp = nc.dram_tensor("wp", [128, W_COLS], _f16, kind="ExternalInput")
    wd = nc.dram_tensor("wd", [128, 1280], _f8, kind="ExternalInput")
    bp = nc.dram_tensor("bp", [128, 9], _f32, kind="ExternalInput")
    yt = nc.dram_tensor("yt", [D_OUT, B_C], _f16, kind="ExternalOutput")

    sig = mybir.ActivationFunctionType.Sigmoid

    with tile.TileContext(nc) as tc:
        with (
            tc.tile_pool(name="consts", bufs=1) as consts,
            tc.tile_pool(name="xp", bufs=4) as xp,
            tc.tile_pool(name="hp", bufs=4) as hp,
            tc.tile_pool(name="yp", bufs=3) as yp,
            tc.tile_pool(name="ps1", bufs=1, space="PSUM") as ps1,
            tc.tile_pool(name="psm", bufs=1, space="PSUM") as psm,
            tc.tile_pool(name="ps5", bufs=2, space="PSUM") as ps5,
        ):
            # warm-up sigmoid with a single dependency: keeps the ACT
            # table-load off the first real (multi-dep) sigmoid
            warm = consts.tile([1, 2], _f32)
            nc.vector.memset(warm[:], 0.0)
            nc.scalar.activation(warm[:, 0:1], warm[:, 0:1], sig, bias=warm[:, 1:2])

            ws = consts.tile([128, W_COLS], _f16)
            wds = consts.tile([128, 1280], _f8)
            bs = consts.tile([128, 9], _f32)

            def w1s(k, m):
                return ws[:, k * 256 + m * 128 : k * 256 + (m + 1) * 128]

            def w2s(k):
                return ws[:, W2_OFF + k * 128 : W2_OFF + (k + 1) * 128]

            xt3 = xt[:].rearrange("(ko p) n -> p ko n", p=128)  # [128, 4, B_C]
            yt3 = yt[:].rearrange("(mo p) n -> p mo n", p=128)  # [128, 4, B_C]

            halves = (slice(0, NT), slice(NT, NP))

            def load_x(t):
                # two [128, 2, NP] tiles: DoubleRow pairs two 128-feature
                # rows per PE cell, contracting 256 features per matmul
                xgs = []
                for g in range(2):
                    xg = xp.tile([128, 2, NP], _f8, tag=f"xg{g}", name=f"xg{g}_{t}")
                    nc.sync.dma_start(xg[:], xt3[:, 2 * g : 2 * g + 2, bass.ts(t, NP)])
                    xgs.append(xg)
                return xgs

            def l1_mms(xgs, m):
                # one m-chunk of L1 for a pair: 4 fp8 DoubleRow matmuls
                # (K=256 each), ACT emitted later
                reg = ps1.tile([128, NP], _f32, tag="p1", name=f"p1_{m}")
                for h in halves:
                    for g in range(2):
                        bb = (g * 2 + m) * 256
                        nc.tensor.matmul(
                            reg[:, h],
                            wds[:, bb : bb + 256].rearrange("p (j mm) -> p j mm", j=2),
                            xgs[g][:, :, h],
                            start=(g == 0), stop=(g == 1),
                            perf_mode=mybir.MatmulPerfMode.DoubleRow,
                        )
                return reg

            def l1_act(reg, m, h1):
                nc.scalar.activation(h1[:, m, :], reg[:], sig, bias=bs[:, m : m + 1])

            def l2_l4(h1):
                p2 = psm.tile([128, NP], _f32, tag="mid")
                for h in halves:
                    nc.tensor.matmul(
                        p2[:, h],
                        wds[:, 1024:1280].rearrange("p (j mm) -> p j mm", j=2),
                        h1[:, :, h],
                        start=True, stop=True,
                        perf_mode=mybir.MatmulPerfMode.DoubleRow,
                    )
                h2 = hp.tile([128, NP], _f16, tag="h2")
                nc.scalar.activation(h2[:], p2[:], sig, bias=bs[:, 2:3])
                return h2

            def l3(h2):
                p3 = psm.tile([128, NT], _f32, tag="mid")
                nc.tensor.matmul(
                    p3[0:64, :], ws[:, W3_OFF : W3_OFF + 64], h2[:, halves[0]],
                    start=True, stop=True, tile_position=(0, 0),
                )
                nc.tensor.matmul(
                    p3[64:128, :], ws[:, W3_OFF + 64 : W3_OFF + 128], h2[:, halves[1]],
                    start=True, stop=True, tile_position=(0, 64),
                )
                h3 = hp.tile([128, NT], _f16, tag="h3")
                nc.scalar.activation(h3[:], p3[:], sig, bias=bs[:, 3:4])
                return h3

            def l4(h3):
                p4 = psm.tile([128, NP], _f32, tag="mid")
                nc.tensor.matmul(
                    p4[:, halves[0]], ws[0:64, W4_OFF : W4_OFF + 128], h3[0:64, :],
                    start=True, stop=True, tile_position=(0, 0),
                )
                nc.tensor.matmul(
                    p4[:, halves[1]], ws[64:128, W4_OFF : W4_OFF + 128], h3[64:128, :],
                    start=True, stop=True, tile_position=(64, 0),
                )
                h4 = hp.tile([128, NP], _f16, tag="h4")
                nc.scalar.activation(h4[:], p4[:], sig, bias=bs[:, 4:5])
                return h4

            def l5_chunk(h4, yts, m):
                p5 = ps5.tile([128, NP], _f32, tag="p5")
                for h in halves:
                    nc.tensor.matmul(
                        p5[:, h],
                        ws[32 * m : 32 * m + 32, W5_OFF + 128 * m : W5_OFF + 128 * (m + 1)],
                        h4[32 * m : 32 * m + 32, h],
                        start=True, stop=True, tile_position=(32 * m, 0),
                    )
                nc.vector.tensor_scalar_add(yts[:, m, :], p5[:], bs[:, 5 + m : 6 + m])

            # ---- software-pipelined emission ----
            # section t runs: tail chain of pair t (L2->L3->L4), L1 matmuls
            # of pair t+1 (PE filler), L5 of pair t-1, with pair t+1's L1
            # ACTs queued last so chain ACTs run with minimal queue delay
            xks = {0: load_x(0)}
            nc.sync.dma_start(wds[:], wd[:])
            nc.sync.dma_start(bs[:], bp[:])
            nc.sync.dma_start(ws[:], wp[:])
            h1s = {}
            h4s = {}
            ytss = {}

            h1s[0] = hp.tile([128, 2, NP], _f8, tag="h1", name="h1_0")
            r0 = l1_mms(xks[0], 0)
            l1_act(r0, 0, h1s[0])
            r1 = l1_mms(xks[0], 1)
            l1_act(r1, 1, h1s[0])
            if N_PAIRS > 1:
                xks[1] = load_x(1)

            for t in range(N_PAIRS):
                if t + 2 < N_PAIRS:
                    xks[t + 2] = load_x(t + 2)

                h2 = l2_l4(h1s.pop(t))

                if t + 1 < N_PAIRS:
                    h1s[t + 1] = hp.tile([128, 2, NP], _f8, tag="h1", name=f"h1_{t+1}")
                    reg0 = l1_mms(xks[t + 1], 0)
                    l1_act(reg0, 0, h1s[t + 1])

                h3 = l3(h2)

                if t + 1 < N_PAIRS:
                    reg1 = l1_mms(xks.pop(t + 1), 1)
                    l1_act(reg1, 1, h1s[t + 1])

                if t - 1 >= 0:
                    l5_chunk(h4s[t - 1], ytss[t - 1], 0)
                    l5_chunk(h4s[t - 1], ytss[t - 1], 1)
                    nc.sync.dma_start(
                        yt3[:, 0:2, bass.ts(t - 1, NP)], ytss[t - 1][:, 0:2, :]
                    )
                    l5_chunk(h4s[t - 1], ytss[t - 1], 2)
                    l5_chunk(h4s.pop(t - 1), ytss[t - 1], 3)
                    nc.sync.dma_start(
                        yt3[:, 2:4, bass.ts(t - 1, NP)], ytss.pop(t - 1)[:, 2:4, :]
                    )

                h4s[t] = l4(h3)

                ytss[t] = yp.tile([128, 4, NP], _f16, tag="yts", name=f"yts_{t}")

            t = N_PAIRS - 1
            l5_chunk(h4s[t], ytss[t], 0)
            nc.sync.dma_start(yt3[:, 0:1, bass.ts(t, NP)], ytss[t][:, 0:1, :])
            l5_chunk(h4s[t], ytss[t], 1)
            nc.sync.dma_start(yt3[:, 1:2, bass.ts(t, NP)], ytss[t][:, 1:2, :])
            l5_chunk(h4s[t], ytss[t], 2)
            nc.sync.dma_start(yt3[:, 2:3, bass.ts(t, NP)], ytss[t][:, 2:3, :])
            l5_chunk(h4s.pop(t), ytss[t], 3)
            nc.sync.dma_start(yt3[:, 3:4, bass.ts(t, NP)], ytss.pop(t)[:, 3:4, :])

    nc.finalize()
    return nc


_NC_CACHE = None


def _get_nc():
    global _NC_CACHE
    if _NC_CACHE is None:
        _NC_CACHE = _build_bass()
    return _NC_CACHE


def _pack_consts(w1, b1, w2, b2, w3, b3, w4, b4, w5, b5):
    wpk = np.zeros((128, W_COLS), dtype=np.float16)
    for k in range(4):
        wpk[:, k * 256 : (k + 1) * 256] = w1.T[k * 128 : (k + 1) * 128, :]
    for k in range(2):
        wpk[:, W2_OFF + k * 128 : W2_OFF + (k + 1) * 128] = w2.T[k * 128 : (k + 1) * 128, :]
    wpk[:, W3_OFF : W3_OFF + 64] = w3.T
    wpk[:, W3_OFF + 64 : W3_OFF + 128] = w3.T
    w4rep = np.tile(w4.T, (1, 4))  # [64, 128]
    wpk[0:64, W4_OFF : W4_OFF + 128] = w4rep
    wpk[64:128, W4_OFF : W4_OFF + 128] = w4rep
    for m in range(4):
        wpk[32 * m : 32 * m + 32, W5_OFF + 128 * m : W5_OFF + 128 * (m + 1)] = w5.T[
            :, 128 * m : 128 * (m + 1)
        ]

    bpk = np.zeros((128, 9), dtype=np.float32)
    bpk[:, 0] = b1[:128]
    bpk[:, 1] = b1[128:]
    bpk[:, 2] = b2
    bpk[0:64, 3] = b3
    bpk[64:128, 3] = b3
    bpk[:, 4] = np.tile(b4, 4)
    for m in range(4):
        bpk[:, 5 + m] = b5[m * 128 : (m + 1) * 128]
    return np.ascontiguousarray(wpk), np.ascontiguousarray(bpk)


def _pack_w1_dr(w1, w2):
    wdk = np.zeros((128, 1280), dtype=np.float32)
    for g in range(2):
        for m in range(2):
            base = (g * 2 + m) * 256
            for j in range(2):
                wdk[:, base + j * 128 : base + (j + 1) * 128] = w1[
                    m * 128 : (m + 1) * 128, g * 256 + j * 128 : g * 256 + (j + 1) * 128
                ].T
    for j in range(2):
        wdk[:, 1024 + j * 128 : 1024 + (j + 1) * 128] = w2[:, j * 128 : (j + 1) * 128].T
    return np.ascontiguousarray(wdk.astype(ml_dtypes.float8_e4m3fn))


def _make_in_maps(x, w1, b1, w2, b2, w3, b3, w4, b4, w5, b5):
    wpk, bpk = _pack_consts(w1, b1, w2, b2, w3, b3, w4, b4, w5, b5)
    shared = {"wp": wpk, "bp": bpk, "wd": _pack_w1_dr(w1, w2)}
    in_maps = []
    for c in range(N_CORES):
        shard = x[c * B_C : (c + 1) * B_C]  # [B_C, 512]
        xtc = np.ascontiguousarray(shard.T.astype(ml_dtypes.float8_e4m3fn))
        in_maps.append({"xt": xtc, **shared})
    return in_maps


def _postprocess(x, results):
    y = np.empty((BATCH, D_OUT), dtype=np.float32)
    for c in range(N_CORES):
        y[c * B_C : (c + 1) * B_C] = results[c]["yt"].T.astype(np.float32)
    # reference: out[:, :in_size] = y, rest zero, in_size = count_nonzero(x[0])
    in_size = int(np.count_nonzero(x[0]))
    if in_size < D_OUT:
        y[:, in_size:] = 0.0
    return y


def run_traced(inputs, trace=False):
    """Run on 8 cores; returns (y_full, BassKernelResults)."""
    nc = _get_nc()
    in_maps = _make_in_maps(**inputs)
    res = run_bass_kernel_spmd(nc, in_maps, core_ids=list(range(N_CORES)), trace=trace)
    y = _postprocess(inputs["x"], res.results)
    return y, res


def kernel(**inputs) -> np.ndarray:
    y, _ = run_traced(inputs, trace=False)
    return y
